# revision 103
# baseline (speedup 1.0000x reference)
"""Trainium2 Bass kernel for nn_MDSFF (deformable-sampling sparse attention).

Math restructuring (same algebra as the fp32r baseline, retuned for engine
balance):
  - Offsets are tanh-bounded to +-1 px, so bilinear grid-sample == 9-tap
    local stencil with per-pixel weights; w_{+1} = relu(t), w_{-1} =
    relu(-t), w_0 = 1 - |t| after clipping t only at image-edge rows/cols.
  - 1x1 convs commute with spatial shifts, so the [B,K,C,H,W] sampled
    tensor is never materialized: sim and the output combine use shifted
    views of A = k_w @ x_aux and Ao = out_w @ x_aux.
  - sim[(k,h)] = sum_t WW_t[(k)] * S_t[(h)], S_t = per-head sum of
    q * shift_t(A); G_t = sum_k WW_t*wk via a 0.25-matmul that also
    broadcasts to 128 partitions; final = sum_t G_t * shift_t(Ao).

Performance structure (vs the fp32r baseline):
  - bf16 everywhere on SBUF: DVE tensor ops hit the 2x 2-byte mode, DMA
    bytes halve, matmuls run 1 cyc/col even for <256-col streams.
  - The 3x3 offset conv runs in fp8e4m3 with DoubleRow perf mode (0.5
    cyc/col, 2 contraction rows per partition) using a scaled-residual
    3-pass scheme -- w8@x8 + w8@xr + wr@x8 with weights pre-scaled by CSC
    to dodge fp8 denormals -- which is both cheaper than bf16 (1.5 vs 2
    cyc/col) and more accurate (~0.14% rms).
  - The 3 dx-taps of each dy row are computed in single wide DVE ops via
    overlapping stride-1 access patterns (A/Ao shifted views differ by one
    column), cutting op counts and sync hops 3x for the map algebra.
  - Edge clips/masks are O(rows) ops on border rows/cols only; y-bounds
    come from per-core scalars so one program serves all 8 cores.
  - Work is spread across DVE/Act/Pool via per-op assignment tables
    (gpsimd cannot touch PSUM, so PSUM evacuations stay on Act/DVE).
  - Emission order software-pipelines the two chunks (chunk-0 S-loop
    outranks chunk-1's conv front; G stages interleave with the next
    chunk's S loop) under the readiness-based tile scheduler.

Sharding: 8 cores = 4 batches x 2 H-halves, 2 column chunks of 16 rows.
Host (numpy) does only data movement and dtype conversion.
"""

import sys

sys.path.insert(0, "/opt/trn_rl_repo")

import numpy as np
import ml_dtypes

import concourse.bass as bass
import concourse.mybir as mybir
from concourse import tile
from concourse.bass_utils import run_bass_kernel_spmd

# ---------------- problem constants (hardcoded per contract) ----------------
B, C, H, W = 4, 256, 64, 64
K = 8
NCORES = 8
ROWS = 32          # center rows per core
CHR = 16           # rows per chunk
N1 = CHR * W       # 1024 center pixels per chunk
HR = 34            # haloed rows per core
XA_W = HR * W      # 2176
XM_W = 2 + 66 * HR  # 2246->2248 padded x_main width: col = 2 + 66*r + w
XM_W = 2248
AW = 2 + 18 * W    # 1154: per-chunk A/Ao width, data cols [1, 1153)
TAPS = [(dy, dx) for dy in (-1, 0, 1) for dx in (-1, 0, 1)]

F32 = mybir.dt.float32
BF16 = mybir.dt.bfloat16
F8 = mybir.dt.float8e4
AF = mybir.ActivationFunctionType
OP = mybir.AluOpType
DR = mybir.MatmulPerfMode.DoubleRow

# mcat column map
MC_I32, MC_HS, MC_AVG4, MC_QB = 0, 32, 64, 96
MC_OFFB = 224      # [16, 1] tanh bias (conv channels, x/y interleaved-perm)
MC_SEL8 = 226      # [16, 64] off->T2 replication selector (x cols 0-31)
MC_W = 290
CSC = 32.0         # conv weight pre-scale (fp8 residual path)
# wcat column map
WC_QW, WC_KW, WC_OW, WC_SEL, WC_I128 = 0, 512, 1024, 1536, 1600
WC_W = 1728

# ------------- engine assignment tables (perf-tuning knobs) ---------------
# M / Fv taps routed to Pool (rest DVE); evac engines per stage
# (A=Act, V=DVE, P=Pool).
M_POOL = {3, 4}
FV_POOL = {(0, 0, 1), (0, 1, 1), (1, 0, 1)}
S_EVAC = ["A"] * 9
S_EVAC_J1 = ["A", "V", "A", "A", "V", "A", "A", "V", "A"]
GB_EVAC = ["A", "A", "A", "A", "V", "A", "A", "A", "A"]
AO_EVAC = ["A", "A", "V"] * 4
Q_EVAC = ["A"] * 4
WW_POOL = {1, 2}
QT_POOL = {(0, 1), (1, 1)}

# packed const blob (bf16 columns): [wcat+A-weights | small consts]
WB_CWP = WC_W          # 288 bf16 cols = 576 fp8
WB_CWR = WB_CWP + 288  # 144 bf16 cols = 288 fp8
WB_YCL = WB_CWR + 144  # 8 bf16 cols = 4 fp32 (rows 0:32)
WB_MC = WB_YCL + 8     # mcat [32, MC_W]
WB_W = WB_MC + MC_W

_CACHE = {}


# ============================ program builder ===============================

def _build_program():
    MAX_WAITS = 1

    SPLIT_OK = {
        "InstDrain", "InstNoOp", "InstMatmult", "InstLdweights",
        "InstTensorTensor", "InstActivation", "InstTensorScalarPtr",
        "InstTensorReduce", "InstCopy", "InstMemSet", "InstMemset",
        "InstReciprocal", "InstTensorTensorReduce", "InstTensorCopy",
    }

    def split_waits(nc):
        # walrus in this container rejects instructions carrying more than
        # MAX_WAITS semaphore waits; spill extras onto same-engine nops.
        # Only safe for engine-FIFO instructions: hoisting a DMA descriptor's
        # wait onto the SP sequencer can deadlock.
        f = nc.m.functions[0]
        for bb in f.blocks:
            insts = bb.instructions
            out = []
            changed = False
            for inst in insts:
                si = inst.sync_info
                waits = list(si.on_wait) if si and si.on_wait else []
                if (len(waits) > MAX_WAITS
                        and type(inst).__name__ in SPLIT_OK
                        and all(w.wait_reg is None for w in waits)):
                    changed = True
                    rest, keep = waits[:-MAX_WAITS], waits[-MAX_WAITS:]
                    for i in range(0, len(rest), MAX_WAITS):
                        nop = mybir.InstNoOp(
                            name=f"Wspill_{inst.name}_{i}", ins=[], outs=[])
                        nop.engine = inst.engine
                        nop.sync_info = mybir.SyncInfo(
                            on_wait=rest[i : i + MAX_WAITS], on_update=[])
                        nc.register_instruction(nop)
                        out.append(nop)
                    inst.sync_info = mybir.SyncInfo(
                        on_wait=keep, on_update=list(si.on_update or [])
                    )
                out.append(inst)
            if changed:
                bb.instructions = out

    nc = bass.Bass("TRN2", target_bir_lowering=False, debug=False,
                   num_devices=NCORES)

    dp = nc.dram_tensor
    xq_d = dp("xq", [128, 2, 2, XM_W], F8, kind="ExternalInput")
    xm16_d = dp("xm16", [128, 2, ROWS * W], BF16, kind="ExternalInput")
    xa_d = dp("xa", [128, 2, XA_W], BF16, kind="ExternalInput")
    wcons_d = dp("wcons", [128, WB_W], BF16, kind="ExternalInput")
    y_d = dp("y", [128, 2, 2, N1], BF16, kind="ExternalOutput")

    V = nc.vector
    A_ = nc.scalar
    P_ = nc.gpsimd

    def mm(out, lhsT, rhs, start, stop, perf_mode=None):
        nc.tensor.matmul(out=out, lhsT=lhsT, rhs=rhs, start=start, stop=stop,
                         perf_mode=perf_mode, skip_group_check=True)

    def evac(eng, out, in_):
        if eng == "A":
            A_.activation(out=out, in_=in_, func=AF.Copy)
        elif eng == "V":
            V.tensor_copy(out=out, in_=in_)
        else:
            P_.tensor_copy(out=out, in_=in_)

    def tt(eng, out, in0, in1, op=OP.mult):
        (P_ if eng == "P" else V).tensor_tensor(out=out, in0=in0, in1=in1,
                                                op=op)

    def dx3(apview, n):
        # [P, 3, n] overlapping view: dx in {-1, 0, +1} at column stride 1
        c = apview.unsqueeze(1).broadcast_to([apview.shape[0], 3, n]).copy()
        c.ap[1] = [1, 3]
        return c

    with tile.TileContext(nc) as tc:
        with (
            nc.allow_low_precision(reason="bf16 pipeline: rounding is within "
                                   "this kernel's error budget"),
            tc.tile_pool(name="pw", bufs=1) as pw,       # weights/consts
            tc.tile_pool(name="pio", bufs=1) as pio,     # inputs
            tc.tile_pool(name="pbig", bufs=1) as pbig,   # q/A/Ao/M/Gb/Fv
            tc.tile_pool(name="pmap", bufs=1) as pmap,   # 32-row maps
            tc.tile_pool(name="psp", bufs=1, space="PSUM") as psp,
        ):
            xq = pio.tile([128, 2, 2, XM_W], F8, tag="xq")
            xm16 = pio.tile([128, 2, ROWS * W], BF16, tag="xm16")
            xa = pio.tile([128, 2, XA_W], BF16, tag="xa")
            wcat_t = pw.tile([128, WC_W], BF16, tag="wcat")
            cwc = pw.tile([128, WB_W - WB_CWP], BF16, tag="cwc")
            XQA = 2 + 66 * 8   # first conv group's rows
            XQ0 = 2 + 66 * 19  # cols holding chunk-0's haloed conv rows
            nc.sync.dma_start(out=cwc[:], in_=wcons_d[:, WB_CWP:])
            nc.sync.dma_start(out=xq[:, :, :, 0:XQA], in_=xq_d[:, :, :, 0:XQA])
            nc.sync.dma_start(out=xq[:, :, :, XQA:XQ0],
                              in_=xq_d[:, :, :, XQA:XQ0])
            nc.sync.dma_start(out=wcat_t[:], in_=wcons_d[:, 0:WC_W])
            for cb in range(2):
                nc.sync.dma_start(out=xa[:, cb, :], in_=xa_d[:, cb, :])
            for cb in range(2):
                nc.sync.dma_start(out=xm16[:, cb, :], in_=xm16_d[:, cb, :])
            nc.sync.dma_start(out=xq[:, :, :, XQ0:], in_=xq_d[:, :, :, XQ0:])
            wcat = wcat_t[:, :]
            cwp = (cwc[:, 0:288].bitcast(F8)
                   .rearrange("p (t cb kt o) -> p t cb kt o", t=9, cb=2, kt=2))
            cwr = (cwc[:, 288:432].bitcast(F8)
                   .rearrange("p (t cb o) -> p t cb o", t=9, cb=2))
            mcat = cwc[0:32, WB_MC - WB_CWP : WB_MC - WB_CWP + MC_W]
            ycl = cwc[0:32, WB_YCL - WB_CWP : WB_YCL - WB_CWP + 8].bitcast(F32)

            def w4(o):  # [128, 2, 2, 128] block at col o
                return wcat[:, o : o + 512].rearrange(
                    "p (cb ob m) -> p cb ob m", cb=2, ob=2)

            qwT, kwT, owT = w4(WC_QW), w4(WC_KW), w4(WC_OW)
            sel = wcat[:, WC_SEL : WC_SEL + 64].rearrange(
                "p (cb j) -> p cb j", cb=2)
            i128 = wcat[:, WC_I128 : WC_I128 + 128]
            i32 = mcat[:, MC_I32 : MC_I32 + 32]
            hs = mcat[:, MC_HS : MC_HS + 32]
            avg4 = mcat[:, MC_AVG4 : MC_AVG4 + 32]
            qb = mcat[:, MC_QB : MC_QB + 128]
            offb = mcat[0:16, MC_OFFB : MC_OFFB + 1]
            yb = ycl

            def xq_view(ch, g, dy, dx, cb=None):
                # [128, 2, 264] fp8 view: 4 haloed rows (66-col padded,
                # contiguous) at local row (1 + 16*ch + 4*g + dy), shift dx.
                # cb=None: ktile dim = cb over the x8 plane (w-resid pass);
                # else: ktile dim = (x8, xr) of channel block cb.
                o = 2 + 66 * (1 + 16 * ch + 4 * g + dy) + dx
                if cb is None:
                    return xq[:, :, 0, o : o + 264]
                return xq[:, cb, :, o : o + 264]

            # per-chunk tiles
            q_sb, A_sb, Ao_sb, T2 = {}, {}, {}, {}
            WWs, E_sb, Ff_sb, WK_sb = {}, {}, {}, {}

            # ---------------- fronts ----------------
            for ch in range(2):
                # offset conv: fp8 DoubleRow, scaled-residual 3-pass
                # (w8@x8 + w8@xr ktile-packed per cb, then wr@x8 cb-packed);
                # tanh(in/CSC + b) evacuates to off, T2 built by replicating
                # DMA (partitions (k,h) <- conv channel k).
                off = pmap.tile([16, N1], BF16, tag="off", bufs=2,
                                name=f"off{ch}")
                T2[ch] = pmap.tile([32, 2, N1], BF16, tag="T2", bufs=2,
                                   name=f"T2{ch}")
                for g in range(4):
                    cps = psp.tile([128, 512], F32, tag="pA", bufs=2,
                                   name=f"cps{ch}{g}")
                    for t in range(9):
                        dy, dx = TAPS[t]
                        for cb in range(2):
                            mm(cps[0:16, 0:264], cwp[:, t, cb, :, :],
                               xq_view(ch, g, dy, dx, cb),
                               start=(t == 0 and cb == 0), stop=False,
                               perf_mode=DR)
                        mm(cps[0:16, 0:264], cwr[:, t, :, :],
                           xq_view(ch, g, dy, dx),
                           start=False, stop=(t == 8), perf_mode=DR)
                    cin = (cps[0:16, 0:264]
                           .rearrange("p (r w) -> p r w", w=66)[:, :, 0:64])
                    tout = (off[:, 256 * g : 256 * (g + 1)]
                            .rearrange("p (r w) -> p r w", w=64))
                    A_.activation(out=tout, in_=cin, func=AF.Tanh,
                                  bias=offb, scale=1.0 / CSC)
                    if g % 2 == 1:
                        # j-half of off complete -> replicate into T2 early
                        j = g // 2
                        tps = psp.tile([128, 512], F32, tag="pA", bufs=2,
                                       name=f"tps{ch}{j}")
                        mm(tps[0:64, :],
                           mcat[0:16, MC_SEL8 : MC_SEL8 + 64],
                           off[:, 512 * j : 512 * (j + 1)],
                           start=True, stop=True)
                        for xy in range(2):
                            A_.activation(
                                out=T2[ch][:, xy, 512 * j : 512 * (j + 1)],
                                in_=tps[32 * xy : 32 * xy + 32, :],
                                func=AF.Copy)

                # q projection (bf16)
                q_sb[ch] = pbig.tile([128, 2, N1], BF16, tag="q", bufs=2,
                                     name=f"q{ch}")
                for ob in range(2):
                    for i in range(2):
                        qps = psp.tile([128, 512], F32, tag="pA", bufs=2,
                                       name=f"qps{ch}{ob}{i}")
                        rhs_c = 512 * (2 * ch + i)
                        for cb in range(2):
                            mm(qps[:], qwT[:, cb, ob, :],
                               xm16[:, cb, rhs_c : rhs_c + 512],
                               start=(cb == 0), stop=(cb == 1))
                        evac(Q_EVAC[2 * ob + i],
                             q_sb[ch][:, ob, 512 * i : 512 * (i + 1)],
                             qps[:])

                # A / Ao projections (bf16)
                A_sb[ch] = pbig.tile([128, 2, AW], BF16, tag="A", bufs=2,
                                     name=f"A{ch}")
                Ao_sb[ch] = pbig.tile([128, 2, AW], BF16, tag="Ao", bufs=2,
                                      name=f"Ao{ch}")
                for dst in (A_sb[ch], Ao_sb[ch]):
                    V.memset(dst[:, :, 0:1], 0.0)
                    V.memset(dst[:, :, AW - 1 : AW], 0.0)
                ei = 0
                for di, (dst, wT) in enumerate(((A_sb[ch], kwT),
                                                (Ao_sb[ch], owT))):
                    for ob in range(2):
                        for j, sz in ((0, 512), (1, 512), (2, 128)):
                            aps = psp.tile([128, 512], F32, tag="pA", bufs=2,
                                           name=f"aps{ch}{di}{ob}{j}")
                            rc = 64 * CHR * ch + 512 * j
                            for cb in range(2):
                                mm(aps[:, 0:sz], wT[:, cb, ob, :],
                                   xa[:, cb, rc : rc + sz],
                                   start=(cb == 0), stop=(cb == 1))
                            evac(AO_EVAC[ei],
                                 dst[:, ob, 1 + 512 * j : 1 + 512 * j + sz],
                                 aps[:, 0:sz])
                            ei += 1

                # ------------- tap-weight maps (DVE, mostly 4x) -------------
                # W3[:, d+1, xy, :] = weight of tap offset d; j-split so DVE
                # starts as soon as each T2 half lands.
                t2 = T2[ch]
                W3 = pmap.tile([32, 3, 2, N1], BF16, tag="W3", bufs=1,
                               name=f"W3_{ch}")
                for j in range(2):
                    sl = slice(512 * j, 512 * (j + 1))
                    t2x = t2[:, 0, sl].rearrange("p (r w) -> p r w", w=64)
                    # edge clips: x at cols 0/63 (consts), y at first/last
                    # row (per-core scalars; inert bounds elsewhere)
                    V.tensor_scalar_max(out=t2x[:, :, 0:1],
                                        in0=t2x[:, :, 0:1], scalar1=-0.5)
                    V.tensor_scalar_min(out=t2x[:, :, 63:64],
                                        in0=t2x[:, :, 63:64], scalar1=0.5)
                    if j == 0:
                        V.tensor_scalar_max(
                            out=t2[:, 1, 0:64], in0=t2[:, 1, 0:64],
                            scalar1=yb[:, 2 * ch : 2 * ch + 1])
                    else:
                        V.tensor_scalar_min(
                            out=t2[:, 1, N1 - 64 : N1],
                            in0=t2[:, 1, N1 - 64 : N1],
                            scalar1=yb[:, 2 * ch + 1 : 2 * ch + 2])
                    V.tensor_scalar_max(out=W3[:, 2, :, sl], in0=t2[:, :, sl],
                                        scalar1=0.0)
                    V.tensor_scalar(out=W3[:, 0, :, sl], in0=t2[:, :, sl],
                                    scalar1=-1.0, scalar2=0.0, op0=OP.mult,
                                    op1=OP.max)
                    V.tensor_tensor(out=W3[:, 1, :, sl], in0=W3[:, 2, :, sl],
                                    in1=W3[:, 0, :, sl], op=OP.add)
                    V.tensor_scalar(out=W3[:, 1, :, sl], in0=W3[:, 1, :, sl],
                                    scalar1=-1.0, scalar2=1.0, op0=OP.mult,
                                    op1=OP.add)
                    # x edge masks: left tap dead at col 0, right at col 63
                    wm1x = W3[:, 0, 0, sl].rearrange("p (r w) -> p r w", w=64)
                    w1x = W3[:, 2, 0, sl].rearrange("p (r w) -> p r w", w=64)
                    V.memset(wm1x[:, :, 0:1], 0.0)
                    V.memset(w1x[:, :, 63:64], 0.0)

                # WW3[dy][:, dxi, :] = wy[dy] * wx[dx]
                WWs[ch] = []
                for dy in (-1, 0, 1):
                    ww = pmap.tile([32, 3, N1], BF16, tag="WW3", bufs=4,
                                   name=f"WW{ch}{dy}")
                    tt("P" if (dy + 1) in WW_POOL else "V", ww[:],
                       W3[:, dy + 1, 1, None, :].broadcast_to([32, 3, N1]),
                       W3[:, :, 0, :])
                    WWs[ch].append(ww)

            # ------- per-chunk S loop / softmax / combine (interleaved) -------
            sim_tiles, fin_state = {}, {}

            def s_dy(ch, dyi):
                if dyi == 0:
                    sim_tiles[ch] = psp.tile([128, N1], F32, tag="acc",
                                             bufs=2, name=f"sim{ch}")
                sim_ps = sim_tiles[ch][0:32, :]
                o_y = 65 + 64 * (dyi - 1)
                S3 = pmap.tile([32, 3, N1], BF16, tag="S3", bufs=2,
                               name=f"S{ch}{dyi}")
                for dxi in range(3):
                    t = 3 * dyi + dxi
                    o_t = o_y + dxi - 1
                    M = pbig.tile([128, 2, N1], BF16, tag="M", bufs=2,
                                  name=f"M{ch}{t}")
                    tt("P" if t in M_POOL else "V", M[:], q_sb[ch][:],
                       A_sb[ch][:, :, o_t : o_t + N1])
                    for j in range(2):
                        sps = psp.tile([32, 512], F32, tag="psS", bufs=2,
                                       name=f"sps{ch}{t}{j}")
                        for cb in range(2):
                            mm(sps[:], sel[:, cb, :],
                               M[:, cb, 512 * j : 512 * (j + 1)],
                               start=(cb == 0), stop=(cb == 1))
                        evac(S_EVAC[t] if j == 0 else S_EVAC_J1[t],
                             S3[:, dxi, 512 * j : 512 * (j + 1)], sps[:])
                P3 = pmap.tile([32, 3, N1], BF16, tag="P3", bufs=2,
                               name=f"P{ch}{dyi}")
                V.tensor_tensor(out=P3[:], in0=WWs[ch][dyi][:], in1=S3[:],
                                op=OP.mult)
                for dxi in range(3):
                    for j in range(2):
                        sl = slice(512 * j, 512 * (j + 1))
                        mm(sim_ps[:, sl], i32, P3[:, dxi, sl],
                           start=(dyi == 0 and dxi == 0),
                           stop=(dyi == 2 and dxi == 2))

            def softmax(ch):
                sim_ps = sim_tiles[ch][0:32, :]
                E_sb[ch] = pmap.tile([32, N1], BF16, tag="E", bufs=2,
                                     name=f"E{ch}")
                A_.activation(out=E_sb[ch][:], in_=sim_ps[:],
                              func=AF.Exp, scale=0.125)
                Ff_sb[ch] = pmap.tile([32, N1], BF16, tag="Ff", bufs=2,
                                      name=f"Ff{ch}")
                for j in range(2):
                    sl = slice(512 * j, 512 * (j + 1))
                    dps = psp.tile([128, 512], F32, tag="pA", bufs=2,
                                   name=f"dps{ch}{j}")
                    mm(dps[0:32, :], hs, E_sb[ch][:, sl], start=True,
                       stop=True)
                    R_t = pmap.tile([32, 512], BF16, tag="R", bufs=2,
                                    name=f"R{ch}{j}")
                    V.reciprocal(out=R_t[:], in_=dps[0:32, :])
                    V.tensor_tensor(out=Ff_sb[ch][:, sl],
                                    in0=E_sb[ch][:, sl], in1=R_t[:],
                                    op=OP.mult)
                WK_sb[ch] = pmap.tile([32, N1], BF16, tag="WK", bufs=2,
                                      name=f"WK{ch}")
                for j in range(2):
                    sl = slice(512 * j, 512 * (j + 1))
                    wps = psp.tile([128, 512], F32, tag="pA", bufs=2,
                                   name=f"wps{ch}{j}")
                    mm(wps[0:32, :], avg4, Ff_sb[ch][:, sl], start=True,
                       stop=True)
                    A_.activation(out=WK_sb[ch][:, sl], in_=wps[0:32, :],
                                  func=AF.Copy)

            Gbs = {0: [], 1: []}

            def ga_dy(ch, dyi):
                Q3 = pmap.tile([32, 3, N1], BF16, tag="Q3", bufs=2,
                               name=f"Q{ch}{dyi}")
                qeng = "P" if (ch, dyi) in QT_POOL else "V"
                for j in range(2):
                    sl = slice(512 * j, 512 * (j + 1))
                    tt(qeng, Q3[:, :, sl], WWs[ch][dyi][:, :, sl],
                       WK_sb[ch][:, None, sl].broadcast_to([32, 3, 512]))
                Gb3 = pbig.tile([128, 3, N1], BF16, tag="Gb3", bufs=3,
                                name=f"Gb{ch}{dyi}")
                for dxi in range(3):
                    t = 3 * dyi + dxi
                    for j in range(2):
                        sl = slice(512 * j, 512 * (j + 1))
                        gps = psp.tile([128, 512], F32, tag="pA", bufs=2,
                                       name=f"gps{ch}{t}{j}")
                        mm(gps[:], qb, Q3[:, dxi, sl], start=True, stop=True)
                        evac(GB_EVAC[t], Gb3[:, dxi, sl], gps[:])
                Gbs[ch].append(Gb3)

            def gb_dy(ch, ob, dyi, jsplit=False):
                if dyi == 0:
                    fin_state[(ch, ob)] = psp.tile(
                        [128, N1], F32, tag="acc", bufs=2,
                        name=f"fin{ch}{ob}")
                fin = fin_state[(ch, ob)]
                o_y = 65 + 64 * (dyi - 1) - 1
                eng = "P" if (ch, ob, dyi) in FV_POOL else "V"
                Fv3 = pbig.tile([128, 3, N1], BF16, tag="Fv3", bufs=3,
                                name=f"Fv{ch}{dyi}{ob}")
                jr = range(2) if jsplit else [None]
                for jj in jr:
                    sj = slice(0, N1) if jj is None else \
                        slice(512 * jj, 512 * (jj + 1))
                    tt(eng, Fv3[:, :, sj], Gbs[ch][dyi][:, :, sj],
                       dx3(Ao_sb[ch][:, ob, o_y + sj.start :
                                      o_y + sj.start + (sj.stop - sj.start)],
                           sj.stop - sj.start))
                    for dxi in range(3):
                        for j in ([jj] if jsplit else range(2)):
                            sl = slice(512 * j, 512 * (j + 1))
                            mm(fin[:, sl], i128, Fv3[:, dxi, sl],
                               start=(dyi == 0 and dxi == 0),
                               stop=(dyi == 2 and dxi == 2))

            def gb_out(ch, ob):
                fin = fin_state[(ch, ob)]
                for j in range(2):
                    sl = slice(512 * j, 512 * (j + 1))
                    osb = pbig.tile([128, 512], BF16, tag="osb", bufs=4,
                                    name=f"osb{ch}{ob}{j}")
                    A_.activation(out=osb[:], in_=fin[:, sl], func=AF.Copy)
                    nc.gpsimd.dma_start(out=y_d[:, ch, ob, sl], in_=osb[:])

            # explicit cross-chunk interleave for engine-priority balance:
            # chunk-0's S loop outranks chunk-1's front on PE so DVE gets
            # fed while the PE grinds the second conv block.
            front(0)
            s_dy(0, 0)
            front(1)
            for dyi in (1, 2):
                s_dy(0, dyi)
            softmax(0)
            for dyi in range(3):
                ga_dy(0, dyi)
                s_dy(1, dyi)
            softmax(1)
            for dyi in range(3):
                gb_dy(0, 0, dyi, jsplit=True)
                gb_dy(0, 1, dyi, jsplit=True)
                ga_dy(1, dyi)
            gb_out(0, 0)
            gb_out(0, 1)
            for dyi in range(3):
                gb_dy(1, 0, dyi, jsplit=True)
                gb_dy(1, 1, dyi, jsplit=True)
            gb_out(1, 0)
            gb_out(1, 1)

    split_waits(nc)
    return nc


# ============================ host-side prep ===============================

def _consts():
    i32 = np.eye(32, dtype=np.float32)
    hs = np.zeros((32, 32), np.float32)
    avg4 = np.zeros((32, 32), np.float32)
    for i in range(32):
        for j in range(32):
            if i % 4 == j % 4:
                hs[i, j] = 1.0
            if i // 4 == j // 4:
                avg4[i, j] = 0.25
    qb = np.full((32, 128), 0.25, np.float32)
    i128 = np.eye(128, dtype=np.float32)
    sel = np.zeros((128, 2, 32), np.float32)
    for cb in range(2):
        for p in range(128):
            h = (128 * cb + p) // 64
            for j in range(32):
                if j % 4 == h:
                    sel[p, cb, j] = 1.0
    return i32, hs, avg4, qb, i128, sel


def _prep_inputs(x_main, x_aux, offset_w, offset_b, q_w, k_w, out_w):
    i32, hs, avg4, qb, i128, sel = _consts()
    bf16 = ml_dtypes.bfloat16
    f8 = ml_dtypes.float8_e4m3

    def wT(wmat):
        # [128, 2, 2, 128]: lhsT[cin_local, cb, ob, o_local] = w[o, cin]
        r = np.zeros((128, 2, 2, 128), np.float32)
        for cb in range(2):
            for ob in range(2):
                r[:, cb, ob, :] = wmat[128 * ob : 128 * (ob + 1),
                                       128 * cb : 128 * (cb + 1)].T
        return r

    # conv weights: fp8 scaled-residual pair; perm puts x-offset channels at
    # rows 0-7 of off, y at 8-15
    perm = [2 * k for k in range(K)] + [2 * k + 1 for k in range(K)]
    cw = np.zeros((128, 9, 2, 16), np.float32)
    for t, (dy, dx) in enumerate(TAPS):
        for cb in range(2):
            cw[:, t, cb, :] = (CSC * offset_w[perm, 128 * cb : 128 * (cb + 1),
                                              dy + 1, dx + 1]).T
    cw8 = cw.astype(f8).astype(np.float32)
    cwr = (cw - cw8).astype(f8)
    cwp = np.repeat(cw8[:, :, :, None, :], 2, axis=3).astype(f8)

    wcat = np.zeros((128, WC_W), np.float32)
    wcat[:, WC_QW : WC_QW + 512] = wT(q_w).reshape(128, 512)
    wcat[:, WC_KW : WC_KW + 512] = wT(k_w).reshape(128, 512)
    wcat[:, WC_OW : WC_OW + 512] = wT(out_w).reshape(128, 512)
    wcat[:, WC_SEL : WC_SEL + 64] = sel.reshape(128, 64)
    wcat[:, WC_I128 : WC_I128 + 128] = i128

    mcat0 = np.zeros((32, MC_W), np.float32)
    mcat0[:, MC_I32 : MC_I32 + 32] = i32
    mcat0[:, MC_HS : MC_HS + 32] = hs
    mcat0[:, MC_AVG4 : MC_AVG4 + 32] = avg4
    mcat0[:, MC_QB : MC_QB + 128] = qb
    mcat0[0:16, MC_OFFB] = offset_b[perm]
    for jj in range(32):
        mcat0[jj // 4, MC_SEL8 + jj] = 1.0           # tx: off row k
        mcat0[8 + jj // 4, MC_SEL8 + 32 + jj] = 1.0  # ty: off row 8+k

    in_maps = []
    for core in range(NCORES):
        b, half = core // 2, core % 2
        h0 = ROWS * half
        xm = np.zeros((128, 2, XM_W), np.float32)
        xa = np.zeros((128, 2, XA_W), np.float32)
        for r in range(HR):
            g = h0 - 1 + r
            if 0 <= g < H:
                for cb in range(2):
                    xm[:, cb, 2 + 66 * r : 2 + 66 * r + 64] = \
                        x_main[b, 128 * cb : 128 * (cb + 1), g, :]
                    xa[:, cb, 64 * r : 64 * r + 64] = \
                        x_aux[b, 128 * cb : 128 * (cb + 1), g, :]
        xm16 = np.zeros((128, 2, ROWS * W), np.float32)
        for cb in range(2):
            xm16[:, cb, :] = x_main[b, 128 * cb : 128 * (cb + 1),
                                    h0 : h0 + ROWS, :].reshape(128, -1)
        ycl = np.zeros((32, 4), np.float32)
        for ch in range(2):
            top, bot = h0 + CHR * ch, h0 + CHR * ch + CHR - 1
            ycl[:, 2 * ch] = -0.5 if top == 0 else -4.0
            ycl[:, 2 * ch + 1] = 0.5 if bot == H - 1 else 4.0
        x8 = xm.astype(f8).astype(np.float32)
        xq = np.stack([x8, xm - x8], axis=2).astype(f8)  # [128, 2, 2, XM_W]

        wcons = np.zeros((128, 2 * WB_W), np.uint8)
        wcons[:, 0 : 2 * WC_W] = \
            wcat.astype(bf16).view(np.uint8).reshape(128, -1)
        wcons[:, 2 * WB_CWP : 2 * WB_CWP + 576] = \
            cwp.view(np.uint8).reshape(128, -1)
        wcons[:, 2 * WB_CWR : 2 * WB_CWR + 288] = \
            cwr.view(np.uint8).reshape(128, -1)
        wcons[0:32, 2 * WB_YCL : 2 * WB_YCL + 16] = \
            ycl.view(np.uint8).reshape(32, -1)
        wcons[0:32, 2 * WB_MC : 2 * WB_MC + 2 * MC_W] = \
            mcat0.astype(bf16).view(np.uint8).reshape(32, -1)
        in_maps.append(dict(
            xq=xq, xm16=xm16.astype(bf16), xa=xa.astype(bf16),
            wcons=wcons.view(bf16)))
    return in_maps


def kernel(**inputs):
    inputs = {k: np.asarray(v, dtype=np.float32) for k, v in inputs.items()}
    if "nc" not in _CACHE:
        _CACHE["nc"] = _build_program()
    nc = _CACHE["nc"]
    in_maps = _prep_inputs(
        inputs["x_main"], inputs["x_aux"], inputs["offset_w"],
        inputs["offset_b"], inputs["q_w"], inputs["k_w"], inputs["out_w"])
    res = run_bass_kernel_spmd(nc, in_maps, list(range(NCORES))).results

    out = np.zeros((B, C, H, W), np.float32)
    for core in range(NCORES):
        b, half = core // 2, core % 2
        y = np.asarray(res[core]["y"]).astype(np.float32)  # [128, 2, 2, N1]
        for ch in range(2):
            for ob in range(2):
                out[b, 128 * ob : 128 * (ob + 1),
                    ROWS * half + CHR * ch : ROWS * half + CHR * (ch + 1),
                    :] = y[:, ch, ob, :].reshape(128, CHR, W)
    return out
